# revision 1
# baseline (speedup 1.0000x reference)
"""Trainium2 kernel for nn_Graph_41609643163904.

The reference op is a sequential per-cell scatter sweep over a 48x48 grid:
for x in 2..45, y in 2..45 (x outer): read center v, zero it, add v*W[y,x]
to the 5x5 neighborhood.  Every step is linear in the grid, so the whole
sweep is one fixed linear operator M (2304x2304) depending only on the
weights.  We build M on the host (cheap: 1936 rank-1 row updates), then the
device work is a batched matmul out = in @ M^T, data-parallel over the
8192-sample batch across 8 NeuronCores (1024 samples/core), zero comm.

Device-side structure (v2):
  * float32r matmul: full fp32 precision at 1 cycle/row (bf16-rate) on the
    PE array for moving dim >= 256.
  * x-major re-flattening of the grid exposes the sweep's causal cone as
    block sparsity: influence travels at most 2 columns leftward, so in
    x-major order M' is block-banded.  Per 128-wide j-tile only a prefix
    of k-tiles is nonzero (188 of 324 blocks = 58%).
  * out^T orientation: M' 128x128 blocks are the stationary operand, the
    batch (512-wide moving operand) streams through -> one implicit weight
    load per block serves 2 matmuls of 512 rows.
  * The batch shard (9.2 MB) is DMA'd first and stays resident; M' blocks
    stream through a bounded pool in exact consumption order, so the PE
    starts after ~1 MB of DMA instead of after the whole M matrix.
"""

import os

import numpy as np

SIZE = 48
D = 2
K = 5
N = SIZE * SIZE          # 2304
B = 8192
NCORES = 8
BS = B // NCORES         # 1024 samples per core

P = 128
NK = N // P              # 18 k-tiles
NJ = N // P              # 18 j-tiles (128 wide)
MW = 512                 # moving-operand width (max for fp32 PSUM bank)
NM = BS // MW            # 2 m-tiles per core

# Structural nonzero k-tile prefix per 128-wide j-tile (x-major layout).
# Influence reaches at most 2 grid-columns leftward: M'[j,k] == 0 whenever
# jx < kx - 2, so k < 48*(jx_max(t)+3).  Valid for ANY weights.
KPREF = tuple(
    min(NK, -(-(SIZE * ((P * (t + 1) - 1) // SIZE + 3)) // P)) for t in range(NJ)
)
NBLK = sum(KPREF)        # 188


def _build_M(weights: np.ndarray) -> np.ndarray:
    """Compose the 1936 per-cell updates into one (N, N) operator, fp64."""
    M = np.eye(N, dtype=np.float64)
    w = weights.astype(np.float64)
    for x in range(D, SIZE - D):
        for y in range(D, SIZE - D):
            c = y * SIZE + x
            wc = w[y, x]
            rc = M[c].copy()
            for dy in range(-D, D + 1):
                r0 = c + dy * SIZE - D
                wrow = wc[dy + D]
                if dy == 0:
                    M[r0:r0 + D] += np.outer(wrow[:D], rc)
                    M[r0 + D + 1:r0 + K] += np.outer(wrow[D + 1:], rc)
                else:
                    M[r0:r0 + K] += np.outer(wrow, rc)
            M[c] = wc[D, D] * rc
    return M


def _build_device_kernel():
    import concourse.mybir as mybir
    from concourse import bacc
    from concourse.tile import TileContext

    f32 = mybir.dt.float32
    f32r = mybir.dt.float32r

    nc = bacc.Bacc()
    xT = nc.dram_tensor("xT", [N, BS], f32r, kind="ExternalInput")
    mt = nc.dram_tensor("mt", [P, NBLK * P], f32r, kind="ExternalInput")
    outT = nc.dram_tensor("outT", [N, BS], f32, kind="ExternalOutput")

    xT_r = xT.rearrange("(k p) m -> k p m", p=P)

    # host packing order: t-major, k within t -> per-t prefix is one
    # contiguous KPREF[t]*64KB region; fetched as a single DMA each.
    block_start = []
    i = 0
    for t in range(NJ):
        block_start.append(i)
        i += KPREF[t]

    with TileContext(nc) as tc:
        with (
            tc.tile_pool(name="xpool", bufs=1) as xpool,
            tc.tile_pool(name="mpool", bufs=6) as mpool,
            tc.tile_pool(name="opool", bufs=3) as opool,
            tc.tile_pool(name="pspool", bufs=2, space="PSUM") as pspool,
        ):
            # Batch shard: one resident tile per k so matmul deps are precise.
            xtiles = []
            issued = 0

            def issue_x(upto):
                nonlocal issued
                while issued < min(upto, NK):
                    xk = xpool.tile([P, BS], f32r, tag=f"x{issued}",
                                    name=f"x{issued}")
                    # ACT's HWDGE ring: keeps the x stream off the SP ring
                    # that feeds the M-prefix stream.
                    nc.scalar.dma_start(out=xk[:], in_=xT_r[issued])
                    xtiles.append(xk)
                    issued += 1

            # x[k] is first needed by j-tile k-1, so issue x DMAs just in
            # time, interleaved with the M-prefix stream (M goes first).
            for t in range(NJ):
                kp = KPREF[t]
                # whole k-prefix for this j-tile in one DMA; the host packs
                # the 128x128 blocks side by side so this is a plain 2D slice.
                mts = mpool.tile([P, NK * P], f32r, tag="m", name=f"m{t}")
                nc.sync.dma_start(
                    out=mts[:, :kp * P],
                    in_=mt[:, block_start[t] * P:(block_start[t] + kp) * P],
                )
                issue_x(t + 2)
                ot = opool.tile([P, BS], f32, tag="o", name=f"o{t}")
                ps = {
                    m: pspool.tile([P, MW], f32, tag=f"ps{m}",
                                   name=f"ps{t}_{m}")
                    for m in range(NM)
                }
                for k in range(kp):
                    for m in range(NM):
                        nc.tensor.matmul(
                            ps[m][:],
                            lhsT=mts[:, k * P:(k + 1) * P],
                            rhs=xtiles[k][:, m * MW:(m + 1) * MW],
                            start=(k == 0),
                            stop=(k == kp - 1),
                        )
                for m in range(NM):
                    nc.vector.tensor_copy(ot[:, m * MW:(m + 1) * MW],
                                          ps[m][:])
                    # SWDGE: keeps stores off both HWDGE input rings.
                    nc.gpsimd.dma_start(
                        out=outT[t * P:(t + 1) * P, m * MW:(m + 1) * MW],
                        in_=ot[:, m * MW:(m + 1) * MW],
                    )
    if not nc.is_finalized():
        nc.finalize()
    return nc


_XMAJOR_IDX = None


def _xmajor_idx():
    global _XMAJOR_IDX
    if _XMAJOR_IDX is None:
        n = np.arange(N)
        _XMAJOR_IDX = (n % SIZE) * SIZE + n // SIZE
    return _XMAJOR_IDX


def kernel(inputs: np.ndarray, weights: np.ndarray) -> np.ndarray:
    from concourse.bass_utils import run_bass_kernel_spmd

    inputs = np.ascontiguousarray(inputs, dtype=np.float32)
    weights = np.ascontiguousarray(weights, dtype=np.float32)

    # Host: build the composed operator and permute to x-major layout.
    M = _build_M(weights)
    idx = _xmajor_idx()
    MTp = np.ascontiguousarray(M[np.ix_(idx, idx)].T.astype(np.float32))

    blocks = [
        MTp[k * P:(k + 1) * P, t * P:(t + 1) * P]
        for t in range(NJ)
        for k in range(KPREF[t])
    ]
    mt_packed = np.ascontiguousarray(np.concatenate(blocks, axis=1))

    # x-major per-sample flatten, then transpose so k is the leading dim.
    xP = inputs.reshape(B, SIZE, SIZE).transpose(0, 2, 1).reshape(B, N)

    nc = _build_device_kernel()
    in_maps = [
        {
            "xT": np.ascontiguousarray(xP[c * BS:(c + 1) * BS].T),
            "mt": mt_packed,
        }
        for c in range(NCORES)
    ]
    trace = bool(int(os.environ.get("KERNEL_TRACE", "0")))
    res = run_bass_kernel_spmd(
        nc, in_maps, core_ids=list(range(NCORES)), trace=trace
    )
    if trace and res.exec_time_ns is not None:
        print(f"HW exec time: {res.exec_time_ns} ns")
        if res.instructions_and_trace is not None:
            print(f"trace: {res.instructions_and_trace[1]}")

    outP = np.concatenate(
        [res.results[c]["outT"].T for c in range(NCORES)], axis=0
    )
    return np.ascontiguousarray(
        outP.reshape(B, SIZE, SIZE).transpose(0, 2, 1).reshape(B, N)
    )

